# revision 1
# baseline (speedup 1.0000x reference)
"""Trainium2 Bass kernel for nn_DecoderV13 (bilinear grid-sample + MLP decoder).

Self-contained: builds the Bass program, shards the 200k queries across 8
NeuronCores (data-parallel; grids + weights replicated), runs via
run_bass_kernel_spmd, and reassembles the full [N, 4] output.

Strategy notes:
- Grid is rearranged on host to a row-pair layout P[y*W+x] = [cell(y,x),
  cell(y+1,x)] (channels-last, 264 ch/cell incl. sdf_grad + pad) so that all
  four bilinear corners of a query are one contiguous 4224B block -> one
  indirect-DMA descriptor per query (128 queries per gather instruction).
- MLP runs feature-major on the tensor engine in float32r (full-rate fp32).
- The pos_enc @ w_pred1 product is folded into one weight on host (no ReLU
  between w_pos2 and w_pred1).
- Fourier sin/cos: ACT Sin is only accurate on [-pi, pi]; we compute
  w = 2^(k-1) * x exactly (power-of-two broadcast matmul in full fp32), then
  s = w - round(w) (exact, magic-constant round) so sin(2 pi s) = sin(2^k pi x)
  and cos via sin(pi/2 - 2 pi |s|).
"""

import os
import sys

import numpy as np

sys.path.insert(0, "/opt/trn_rl_repo")

from concourse import bacc, bass, mybir, tile  # noqa: E402
from concourse.bass_utils import run_bass_kernel_spmd  # noqa: E402
from concourse.masks import make_identity  # noqa: E402

F32 = mybir.dt.float32
F32R = mybir.dt.float32r
I32 = mybir.dt.int32
Alu = mybir.AluOpType
Act = mybir.ActivationFunctionType

# Problem constants (hardcoded per harness contract).
N_FULL = 200000
NCORES = 8
H, W, C = 256, 512, 256
D = 264  # 256 geo + 2 sdf_grad + 6 pad channels per cell
ROW = 2 * D  # one P row: [cell(y,x) | cell(y+1,x)]
NUM_FREQS = 10
GRID_X_MIN, GRID_X_MAX = -2.0, 4.0
GRID_Y_MIN, GRID_Y_MAX = -1.5, 1.5

NC = 25088  # queries per core (200704 padded / 8)
NPAD = NC * NCORES
CHUNK = 128  # queries per gather instruction / partition dim
NCHUNK = NC // CHUNK  # 196
TCH = 4  # chunks per MLP tile
TOK = CHUNK * TCH  # 512 tokens per MLP tile
NTILE = NCHUNK // TCH  # 49

MAGIC = 12582912.0  # 1.5*2^23: (x + MAGIC) - MAGIC == round-to-nearest-even(x)
TWO_PI = 6.283185307179586
HALF_PI = 1.5707963267948966

# ident row layout ([34, tok] tile): rows 0..8 host dims, rows 9..31 zero-pad,
# rows 32..33 sdf_grad (transposed on device). Maps to reference identity dims.
HOST_IDENT_REF_DIMS = [0, 1, 2, 3, 4, 7, 8, 9, 10]  # pos, uinf, sdf, normals, flow
IDENT_ROWS = 34
SG_ROW = 32  # where sdf_grad.x lands
REF_DIM_TO_ROW = {0: 0, 1: 1, 2: 2, 3: 3, 4: 4, 7: 5, 8: 6, 9: 7, 10: 8, 5: 32, 6: 33}
# rows of ff_sin/ff_cos holding the 110 fourier args (rows 96..104 in ff_sin
# carry the 9 host identity dims; 96..97 in ff_cos carry sdf_grad; both are
# overwritten after the Sin op, so args avoid rows 96..104).
ARG_ROWS = list(range(96)) + list(range(114, 128))


def _install_ntff_shim():
    import contextlib
    import ctypes
    import types

    try:
        import antenv.axon_hooks  # noqa: F401

        return
    except ImportError:
        pass
    so = "/opt/axon/libaxon_pjrt.so"
    hook = None
    if os.path.exists(so):
        lib = ctypes.CDLL(so)
        if hasattr(lib, "axon_start_nrt_profile"):
            lib.axon_start_nrt_profile.argtypes = [
                ctypes.POINTER(ctypes.c_int64),
                ctypes.c_size_t,
            ]
            lib.axon_start_nrt_profile.restype = ctypes.c_int64
            lib.axon_stop_nrt_profile.argtypes = [ctypes.c_char_p]
            lib.axon_stop_nrt_profile.restype = ctypes.c_int64

            @contextlib.contextmanager
            def _hook(output_dir, device_ids):
                import jax

                jax.devices()
                if device_ids:
                    ids = (ctypes.c_int64 * len(device_ids))(*device_ids)
                    rc = lib.axon_start_nrt_profile(ids, len(device_ids))
                else:
                    rc = lib.axon_start_nrt_profile(None, 0)
                if rc != 0:
                    raise RuntimeError(f"axon_start_nrt_profile rc={rc}")
                try:
                    yield
                finally:
                    n = lib.axon_stop_nrt_profile(str(output_dir).encode())
                    print(f"ntff profile: {n} file(s) -> {output_dir}", file=sys.stderr)

            hook = _hook
    mod = types.ModuleType("antenv.axon_hooks")
    mod.get_axon_ntff_profile_hook = lambda: hook
    mod.set_axon_ntff_profile_hook = lambda h: None
    sys.modules["antenv.axon_hooks"] = mod


def _norm_index_pipeline(nc, pool, q_t, vmin, scale2, imax, magic_pool=None):
    """From physical coords [128, NCHUNK] produce (i0_clamped, frac_w) exactly
    mirroring the reference's fp32 normalize->clip->scale->floor chain."""
    a = pool.tile([CHUNK, NCHUNK], F32, tag=f"ix_a{vmin}")
    # 2*(q - vmin)/span - 1, folded as (q + (-vmin)) * (2/span)
    nc.vector.tensor_scalar(a[:], q_t[:], -vmin, scale2, op0=Alu.add, op1=Alu.mult)
    b = pool.tile([CHUNK, NCHUNK], F32, tag=f"ix_b{vmin}")
    nc.vector.tensor_scalar(b[:], a[:], 1.0, -1.0, op0=Alu.subtract, op1=Alu.max)
    c = pool.tile([CHUNK, NCHUNK], F32, tag=f"ix_c{vmin}")
    nc.vector.tensor_scalar(c[:], b[:], 1.0, 1.0, op0=Alu.min, op1=Alu.add)
    ix = pool.tile([CHUNK, NCHUNK], F32, tag=f"ix_i{vmin}")
    nc.vector.tensor_scalar(ix[:], c[:], 0.5, float(imax), op0=Alu.mult, op1=Alu.mult)
    f = pool.tile([CHUNK, NCHUNK], F32, tag=f"ix_f{vmin}")
    nc.vector.tensor_scalar(f[:], ix[:], MAGIC, MAGIC, op0=Alu.add, op1=Alu.subtract)
    m = pool.tile([CHUNK, NCHUNK], F32, tag=f"ix_m{vmin}")
    nc.vector.tensor_tensor(m[:], f[:], ix[:], op=Alu.is_gt)
    g = pool.tile([CHUNK, NCHUNK], F32, tag=f"ix_g{vmin}")
    nc.vector.tensor_tensor(g[:], f[:], m[:], op=Alu.subtract)  # floor(ix)
    i0 = pool.tile([CHUNK, NCHUNK], F32, tag=f"ix_0{vmin}")
    nc.vector.tensor_scalar(i0[:], g[:], 0.0, float(imax - 1), op0=Alu.max, op1=Alu.min)
    wf = pool.tile([CHUNK, NCHUNK], F32, tag=f"ix_w{vmin}")
    nc.vector.tensor_tensor(wf[:], ix[:], i0[:], op=Alu.subtract)
    return i0, wf


def build_kernel():
    nc = bacc.Bacc("TRN2", target_bir_lowering=False, debug=False, num_devices=NCORES)

    p_d = nc.dram_tensor("p_grid", [H * W, ROW], F32, kind="ExternalInput")
    xq_d = nc.dram_tensor("xq", [CHUNK, NCHUNK], F32, kind="ExternalInput")
    yq_d = nc.dram_tensor("yq", [CHUNK, NCHUNK], F32, kind="ExternalInput")
    ident9_d = nc.dram_tensor("ident9", [9, NC], F32, kind="ExternalInput")
    identhl_d = nc.dram_tensor("identhl", [18, NC], F32R, kind="ExternalInput")
    bmat_d = nc.dram_tensor("bmat", [IDENT_ROWS, 128], F32, kind="ExternalInput")
    w1sin_d = nc.dram_tensor("w1sin", [128, 256], F32R, kind="ExternalInput")
    w1cos_d = nc.dram_tensor("w1cos", [128, 256], F32R, kind="ExternalInput")
    w1a_d = nc.dram_tensor("w1a", [256, 256], F32R, kind="ExternalInput")
    wfold_d = nc.dram_tensor("wfold", [256, 256], F32R, kind="ExternalInput")
    w2_d = nc.dram_tensor("w2", [256, 4], F32R, kind="ExternalInput")
    b1_d = nc.dram_tensor("b1", [256, 1], F32, kind="ExternalInput")
    bfold_d = nc.dram_tensor("bfold", [256, 1], F32, kind="ExternalInput")
    b2_d = nc.dram_tensor("b2", [4, 1], F32, kind="ExternalInput")
    out_d = nc.dram_tensor("out", [4, NC], F32, kind="ExternalOutput")
    debug = bool(int(os.environ.get("KERNEL_DEBUG", "0")))
    DBG_TILE = int(os.environ.get("KERNEL_DEBUG_TILE", "0"))
    if debug:
        ffs_dump = nc.dram_tensor("ffs_dump", [128, TOK], F32, kind="ExternalOutput")
        ffc_dump = nc.dram_tensor("ffc_dump", [128, TOK], F32, kind="ExternalOutput")
        h_dump = nc.dram_tensor("h_dump", [256, TOK], F32, kind="ExternalOutput")
        id_dump = nc.dram_tensor("id_dump", [IDENT_ROWS, TOK], F32, kind="ExternalOutput")
        s_dump = nc.dram_tensor("s_dump", [128, TOK], F32, kind="ExternalOutput")
        lg_dump = nc.dram_tensor("lg_dump", [256, TOK], F32, kind="ExternalOutput")

    from contextlib import ExitStack

    with tile.TileContext(nc) as tc, ExitStack() as ctx:
        cpool = ctx.enter_context(tc.tile_pool(name="const", bufs=1))
        ipool = ctx.enter_context(tc.tile_pool(name="idx", bufs=1))
        gpool = ctx.enter_context(tc.tile_pool(name="g", bufs=4))
        lpool = ctx.enter_context(tc.tile_pool(name="lg", bufs=8))
        fpool = ctx.enter_context(tc.tile_pool(name="fm", bufs=2))
        apool = ctx.enter_context(tc.tile_pool(name="act", bufs=2))
        psA = ctx.enter_context(tc.tile_pool(name="psA", bufs=1, space="PSUM"))
        psB = ctx.enter_context(tc.tile_pool(name="psB", bufs=1, space="PSUM"))

        # ---- constants ----
        ident128 = cpool.tile([128, 128], F32, tag="ident128")
        make_identity(nc, ident128[:])
        bmat_t = cpool.tile([IDENT_ROWS, 128], F32, tag="bmat")
        nc.sync.dma_start(bmat_t[:], bmat_d[:])
        w1sin_t = cpool.tile([128, 256], F32R, tag="w1sin")
        nc.sync.dma_start(w1sin_t[:], w1sin_d[:])
        w1cos_t = cpool.tile([128, 256], F32R, tag="w1cos")
        nc.sync.dma_start(w1cos_t[:], w1cos_d[:])
        w1a_t = []
        wfold_t = []
        w2_t = []
        for kt in range(2):
            t = cpool.tile([128, 256], F32R, tag=f"w1a{kt}", name=f"w1a_t{kt}")
            nc.sync.dma_start(t[:], w1a_d[kt * 128 : (kt + 1) * 128, :])
            w1a_t.append(t)
            t = cpool.tile([128, 256], F32R, tag=f"wfold{kt}", name=f"wfold_t{kt}")
            nc.sync.dma_start(t[:], wfold_d[kt * 128 : (kt + 1) * 128, :])
            wfold_t.append(t)
            t = cpool.tile([128, 4], F32R, tag=f"w2{kt}", name=f"w2_t{kt}")
            nc.sync.dma_start(t[:], w2_d[kt * 128 : (kt + 1) * 128, :])
            w2_t.append(t)
        b1_t = cpool.tile([128, 2], F32, tag="b1")
        nc.sync.dma_start(b1_t[:, 0:1], b1_d[0:128, :])
        nc.sync.dma_start(b1_t[:, 1:2], b1_d[128:256, :])
        bfold_t = cpool.tile([128, 2], F32, tag="bfold")
        nc.sync.dma_start(bfold_t[:, 0:1], bfold_d[0:128, :])
        nc.sync.dma_start(bfold_t[:, 1:2], bfold_d[128:256, :])
        b2_t = cpool.tile([4, 1], F32, tag="b2")
        nc.sync.dma_start(b2_t[:], b2_d[:])
        halfpi_t = cpool.tile([128, 1], F32, tag="halfpi")
        nc.vector.memset(halfpi_t[:], HALF_PI)

        # persistent double-buffered ident tiles (rows 9..31 stay zero forever)
        ident_ts = []
        for par in range(2):
            it = cpool.tile([IDENT_ROWS, TOK], F32, tag=f"ident{par}", name=f"ident_t{par}")
            nc.vector.memset(it[0:32, :], 0.0)
            ident_ts.append(it)

        # ---- index pipeline (whole core at once) ----
        xq_t = ipool.tile([CHUNK, NCHUNK], F32, tag="xq")
        nc.sync.dma_start(xq_t[:], xq_d[:])
        yq_t = ipool.tile([CHUNK, NCHUNK], F32, tag="yq")
        nc.sync.dma_start(yq_t[:], yq_d[:])

        x0, wx = _norm_index_pipeline(nc, ipool, xq_t, GRID_X_MIN, 1.0 / 3.0, W - 1)
        y0, wy = _norm_index_pipeline(nc, ipool, yq_t, GRID_Y_MIN, 2.0 / 3.0, H - 1)

        idxf = ipool.tile([CHUNK, NCHUNK], F32, tag="idxf")
        nc.vector.scalar_tensor_tensor(
            idxf[:], y0[:], float(W), x0[:], op0=Alu.mult, op1=Alu.add
        )
        idx_t = ipool.tile([CHUNK, NCHUNK], I32, tag="idx")
        nc.vector.tensor_copy(idx_t[:], idxf[:])

        onemwx = ipool.tile([CHUNK, NCHUNK], F32, tag="onemwx")
        nc.vector.tensor_scalar(onemwx[:], wx[:], -1.0, 1.0, op0=Alu.mult, op1=Alu.add)
        onemwy = ipool.tile([CHUNK, NCHUNK], F32, tag="onemwy")
        nc.vector.tensor_scalar(onemwy[:], wy[:], -1.0, 1.0, op0=Alu.mult, op1=Alu.add)
        w00 = ipool.tile([CHUNK, NCHUNK], F32, tag="w00")
        nc.vector.tensor_tensor(w00[:], onemwx[:], onemwy[:], op=Alu.mult)
        w01 = ipool.tile([CHUNK, NCHUNK], F32, tag="w01")
        nc.vector.tensor_tensor(w01[:], wx[:], onemwy[:], op=Alu.mult)
        w10 = ipool.tile([CHUNK, NCHUNK], F32, tag="w10")
        nc.vector.tensor_tensor(w10[:], onemwx[:], wy[:], op=Alu.mult)
        w11 = ipool.tile([CHUNK, NCHUNK], F32, tag="w11")
        nc.vector.tensor_tensor(w11[:], wx[:], wy[:], op=Alu.mult)

        # ---- main loop over MLP tiles ----
        for t in range(NTILE):
            ident_t = ident_ts[t % 2]
            nc.sync.dma_start(
                ident_t[0:9, :], ident9_d[:, t * TOK : (t + 1) * TOK]
            )

            lg_ps = [psA.tile([128, TOK], F32, tag=f"lg_ps{h}", name=f"lg_ps{h}_{t}") for h in range(2)]
            sg_ps = psB.tile([4, TOK], F32, tag="small", name=f"sg_ps_{t}")

            for j in range(TCH):
                ch = t * TCH + j
                g_t = gpool.tile([CHUNK, 4 * D], F32, tag="g")
                nc.gpsimd.indirect_dma_start(
                    out=g_t[:],
                    out_offset=None,
                    in_=p_d[:],
                    in_offset=bass.IndirectOffsetOnAxis(
                        ap=idx_t[:, ch : ch + 1], axis=0
                    ),
                )
                # G per partition: [v00 | v10 | v01 | v11] (264 each)
                lg = lpool.tile([CHUNK, D], F32, tag=f"lg{j}")
                nc.vector.tensor_scalar(
                    lg[:], g_t[:, 0:D], w00[:, ch : ch + 1], None, op0=Alu.mult
                )
                nc.vector.scalar_tensor_tensor(
                    lg[:], g_t[:, D : 2 * D], w10[:, ch : ch + 1], lg[:],
                    op0=Alu.mult, op1=Alu.add,
                )
                nc.vector.scalar_tensor_tensor(
                    lg[:], g_t[:, 2 * D : 3 * D], w01[:, ch : ch + 1], lg[:],
                    op0=Alu.mult, op1=Alu.add,
                )
                nc.vector.scalar_tensor_tensor(
                    lg[:], g_t[:, 3 * D : 4 * D], w11[:, ch : ch + 1], lg[:],
                    op0=Alu.mult, op1=Alu.add,
                )
                # transpose to feature-major
                for h in range(2):
                    nc.tensor.transpose(
                        lg_ps[h][:, j * 128 : (j + 1) * 128],
                        lg[:, h * 128 : (h + 1) * 128],
                        ident128[:],
                    )
                nc.tensor.transpose(
                    sg_ps[0:2, j * 128 : (j + 1) * 128], lg[:, 256:258], ident128[:]
                )

            lg_fm = []
            for h in range(2):
                fm = fpool.tile([128, TOK], F32R, tag=f"lgfm{h}", name=f"lgfm{h}_{t}")
                nc.vector.tensor_copy(fm[:], lg_ps[h][:])
                lg_fm.append(fm)
            nc.scalar.activation(ident_t[SG_ROW : SG_ROW + 2, :], sg_ps[0:2, :], Act.Identity)

            # ---- fourier: w = 2^(k-1) x (exact fp32 matmul), s = w - round(w)
            w_ps = psA.tile([128, TOK], F32, tag="w_ps")
            nc.tensor.matmul(
                w_ps[:], lhsT=bmat_t[:], rhs=ident_t[:], start=True, stop=True
            )
            m_t = apool.tile([128, TOK], F32, tag="m_t")
            nc.vector.tensor_scalar(
                m_t[:], w_ps[:], MAGIC, MAGIC, op0=Alu.add, op1=Alu.subtract
            )
            s_t = apool.tile([128, TOK], F32, tag="s_t")
            nc.vector.tensor_tensor(s_t[:], w_ps[:], m_t[:], op=Alu.subtract)
            v_t = apool.tile([128, TOK], F32, tag="v_t")
            nc.scalar.activation(v_t[:], s_t[:], Act.Abs)
            ff_sin = apool.tile([128, TOK], F32R, tag="ff_sin")
            nc.scalar.activation(ff_sin[:], s_t[:], Act.Sin, scale=TWO_PI)
            # rows 96..113: host identity dims, bf16 hi/lo split (f32r-exact)
            nc.sync.dma_start(
                ff_sin[96:114, :], identhl_d[:, t * TOK : (t + 1) * TOK]
            )
            ff_cos = apool.tile([128, TOK], F32R, tag="ff_cos")
            nc.scalar.activation(ff_cos[:], v_t[:], Act.Sin, bias=halfpi_t[:, 0:1], scale=-TWO_PI)
            # rows 96..97: sdf_grad dims (scaled 1/64; weights scaled up 64x)
            nc.scalar.activation(
                ff_cos[96:98, :], sg_ps[0:2, :], Act.Identity, scale=0.015625
            )

            if debug and t == DBG_TILE:
                nc.sync.dma_start(id_dump[:], ident_t[:])
                nc.sync.dma_start(s_dump[:], s_t[:])
                nc.sync.dma_start(ffs_dump[:], ff_sin[:].bitcast(F32))
                nc.sync.dma_start(ffc_dump[:], ff_cos[:].bitcast(F32))
                nc.sync.dma_start(lg_dump[0:128, :], lg_fm[0][:].bitcast(F32))
                nc.sync.dma_start(lg_dump[128:256, :], lg_fm[1][:].bitcast(F32))

            # ---- pos MLP layer 1 (includes fp32 identity chunk) ----
            h_ps = [psA.tile([128, TOK], F32, tag=f"h_ps{mt}", name=f"h_ps{mt}_{t}") for mt in range(2)]
            for mt in range(2):
                msl = slice(mt * 128, (mt + 1) * 128)
                nc.tensor.matmul(
                    h_ps[mt][:], lhsT=w1sin_t[:, msl], rhs=ff_sin[:],
                    start=True, stop=False,
                )
                nc.tensor.matmul(
                    h_ps[mt][:], lhsT=w1cos_t[:, msl], rhs=ff_cos[:],
                    start=False, stop=True,
                )
            h_sb = []
            for mt in range(2):
                hs = apool.tile([128, TOK], F32R, tag=f"h_sb{mt}", name=f"h_sb{mt}_{t}")
                nc.scalar.activation(
                    hs[:], h_ps[mt][:], Act.Relu, bias=b1_t[:, mt : mt + 1]
                )
                h_sb.append(hs)

            # ---- pred layer 1: W1a.T @ local_geo + Wfold.T @ h ----
            p_ps = [psA.tile([128, TOK], F32, tag=f"p_ps{mt}", name=f"p_ps{mt}_{t}") for mt in range(2)]
            for mt in range(2):
                msl = slice(mt * 128, (mt + 1) * 128)
                nc.tensor.matmul(
                    p_ps[mt][:], lhsT=w1a_t[0][:, msl], rhs=lg_fm[0][:],
                    start=True, stop=False,
                )
                nc.tensor.matmul(
                    p_ps[mt][:], lhsT=w1a_t[1][:, msl], rhs=lg_fm[1][:],
                    start=False, stop=False,
                )
                nc.tensor.matmul(
                    p_ps[mt][:], lhsT=wfold_t[0][:, msl], rhs=h_sb[0][:],
                    start=False, stop=False,
                )
                nc.tensor.matmul(
                    p_ps[mt][:], lhsT=wfold_t[1][:, msl], rhs=h_sb[1][:],
                    start=False, stop=True,
                )
            if debug and t == DBG_TILE:
                for mt in range(2):
                    nc.sync.dma_start(h_dump[mt*128:(mt+1)*128, :], h_sb[mt][:].bitcast(F32))
            h2_sb = []
            for mt in range(2):
                hs = apool.tile([128, TOK], F32R, tag=f"h2_sb{mt}", name=f"h2_sb{mt}_{t}")
                nc.scalar.activation(
                    hs[:], p_ps[mt][:], Act.Relu, bias=bfold_t[:, mt : mt + 1]
                )
                h2_sb.append(hs)

            # ---- pred layer 2 ----
            o_ps = psB.tile([4, TOK], F32, tag="small", name=f"o_ps_{t}")
            nc.tensor.matmul(
                o_ps[:], lhsT=w2_t[0][:], rhs=h2_sb[0][:], start=True, stop=False
            )
            nc.tensor.matmul(
                o_ps[:], lhsT=w2_t[1][:], rhs=h2_sb[1][:], start=False, stop=True
            )
            o_sb = apool.tile([4, TOK], F32, tag="o_sb")
            nc.scalar.activation(o_sb[:], o_ps[:], Act.Identity, bias=b2_t[0:4, 0:1])
            nc.sync.dma_start(out_d[:, t * TOK : (t + 1) * TOK], o_sb[:])

    nc.compile()
    return nc


_NC_CACHE = {}


def _get_nc():
    if "nc" not in _NC_CACHE:
        _NC_CACHE["nc"] = build_kernel()
    return _NC_CACHE["nc"]


def _host_prep(processed_grid, sdf_grad_grid, query_pos, query_uinf, query_sdf,
               query_normals, query_flow, w_pos1, b_pos1, w_pos2, b_pos2,
               w_pred1, b_pred1, w_pred2, b_pred2):
    # Row-pair grid: P[y*W+x] = [cell(y,x) | cell(y+1,x)], channels-last.
    cell = np.zeros((H, W, D), dtype=np.float32)
    cell[:, :, :C] = np.asarray(processed_grid[0]).transpose(1, 2, 0)
    cell[:, :, C : C + 2] = np.asarray(sdf_grad_grid[0]).transpose(1, 2, 0)
    P = np.zeros((H, W, ROW), dtype=np.float32)
    P[: H - 1, :, :D] = cell[: H - 1]
    P[: H - 1, :, D:] = cell[1:]
    P = P.reshape(H * W, ROW)

    def pad(a):
        a = np.asarray(a, dtype=np.float32)
        reps = np.repeat(a[-1:], NPAD - N_FULL, axis=0)
        return np.concatenate([a, reps], axis=0)

    qp = pad(query_pos)
    ident9_full = np.stack([
        qp[:, 0], qp[:, 1],
        pad(query_uinf)[:, 0], pad(query_uinf)[:, 1],
        pad(query_sdf)[:, 0],
        pad(query_normals)[:, 0], pad(query_normals)[:, 1],
        pad(query_flow)[:, 0], pad(query_flow)[:, 1],
    ])  # [9, NPAD]

    # B matrix: w-arg columns (ARG_ROWS layout) = 2^(k-1) at the ident row of d.
    bmat = np.zeros((IDENT_ROWS, 128), dtype=np.float32)
    w1sin = np.zeros((128, 256), dtype=np.float32)
    w1cos = np.zeros((128, 256), dtype=np.float32)
    w_pos1 = np.asarray(w_pos1, dtype=np.float32)
    for d in range(11):
        row = REF_DIM_TO_ROW[d]
        for k in range(NUM_FREQS):
            col = d * NUM_FREQS + k
            arow = ARG_ROWS[col]
            bmat[row, arow] = float(2.0 ** (k - 1))
            w1sin[arow, :] = w_pos1[11 + col, :]
            w1cos[arow, :] = w_pos1[121 + col, :]
    # ff_sin rows 96..104 / 105..113 carry bf16 hi/lo of the host identity
    # dims, scaled by 1/64 (f32r noise scales with the largest term in the
    # reduction); weights are scaled up by 64 to compensate.
    for i, d in enumerate(HOST_IDENT_REF_DIMS):
        w1sin[96 + i, :] = w_pos1[d, :] * 64.0
        w1sin[105 + i, :] = w_pos1[d, :] * 64.0
    # ff_cos rows 96..97 carry sdf_grad (scaled 1/64 on device)
    w1cos[96, :] = w_pos1[5, :] * 64.0
    w1cos[97, :] = w_pos1[6, :] * 64.0

    w_pred1 = np.asarray(w_pred1, dtype=np.float64)
    wfold = (np.asarray(w_pos2, dtype=np.float64) @ w_pred1[256:]).astype(np.float32)
    bfold = (np.asarray(b_pred1, dtype=np.float64)
             + np.asarray(b_pos2, dtype=np.float64) @ w_pred1[256:]).astype(np.float32)
    w1a = w_pred1[:256].astype(np.float32)

    per_core = []
    for cidx in range(NCORES):
        sl = slice(cidx * NC, (cidx + 1) * NC)
        xq = np.ascontiguousarray(qp[sl, 0].reshape(NCHUNK, CHUNK).T)
        yq = np.ascontiguousarray(qp[sl, 1].reshape(NCHUNK, CHUNK).T)
        import ml_dtypes
        hi = ident9_full[:, sl].astype(ml_dtypes.bfloat16).astype(np.float32)
        lo = ((ident9_full[:, sl] - hi).astype(ml_dtypes.bfloat16)).astype(np.float32)
        identhl = np.ascontiguousarray(
            np.concatenate([hi, lo], axis=0) * np.float32(1.0 / 64.0))
        per_core.append({
            "p_grid": P,
            "xq": xq,
            "yq": yq,
            "ident9": np.ascontiguousarray(ident9_full[:, sl]),
            "identhl": identhl,
            "bmat": bmat,
            "w1sin": w1sin,
            "w1cos": w1cos,
            "w1a": w1a,
            "wfold": wfold,
            "w2": np.asarray(w_pred2, dtype=np.float32),
            "b1": np.asarray(b_pos1, dtype=np.float32).reshape(256, 1),
            "bfold": bfold.reshape(256, 1),
            "b2": np.asarray(b_pred2, dtype=np.float32).reshape(4, 1),
        })
    return per_core


def kernel(**inputs):
    _install_ntff_shim()
    nc = _get_nc()
    in_maps = _host_prep(**inputs)
    trace = bool(int(os.environ.get("KERNEL_TRACE", "0")))
    res = run_bass_kernel_spmd(
        nc, in_maps, core_ids=list(range(NCORES)), trace=trace
    )
    if trace:
        kernel.last_exec_time_ns = res.exec_time_ns
        kernel.last_results = res
    outs = [res.results[cidx]["out"] for cidx in range(NCORES)]  # [4, NC] each
    full = np.concatenate(outs, axis=1)[:, :N_FULL]  # [4, N]
    return np.ascontiguousarray(full.T)

